# revision 16
# baseline (speedup 1.0000x reference)
"""Trainium2 Bass kernel for nn_EmbeddingLayer (GNN message passing layer).

Reference computation (per batch b):
    x1 = nf @ W1.T                                   (N,D)
    x2 = (adj @ prev) @ W2.T                         (N,D)
    x4 = leaky(ef[...,None] @ W4.T)                  (N,N,D)
    s  = einsum('ij,ijd->id', adj, x4) / rowsum(adj) (N,D)
    x3 = s @ W3.T
    out = leaky(x1 + x2 + x3)

Key algebraic collapse (avoids the (N,N,D) intermediate entirely):
    leaky(e*w) = 0.505*e*w + 0.495*|e|*|w|   (leaky slope 0.01)
    and since adj >= 0 (uniform [0,1) fill):  adj*|e| = |adj*e|
    =>  x3 = r1n (x) u0 + r2n (x) u1         (rank-2 outer product)
        u0 = 0.505*(W3 @ w4), u1 = 0.495*(W3 @ |w4|)
        r1 = rowsum(adj*ef), r2 = rowsum(|adj*ef|), r?n = r?/rowsum(adj)

Sharding: data-parallel, one batch element per NeuronCore (B=8, 8 cores).

Schedule (final): adj/ef use the interleaved row layout (DRAM row 4p+s at
SBUF partition p, slot s) so a 2-slot transfer moves 4KB contiguous DRAM
per partition (4KB descriptors; 2KB descriptors halve the effective HBM
rate).  adjt uses the block layout (row 128s+p) so the fp32 tT chain
contracts sequential 128-row k blocks - closest to the CPU reference's
accumulation order, which is what keeps the pointwise rel-err comfortably
under the gate (scrambled k orders measurably fail it).  Two HWDGE rings
drain concurrently, adjt first so the PE tT chain runs early on the warm
HAM clock:
    SP ring:  adjt slabs 0-3 (block), adj s01, ef s01
    ACT ring: packed smalls, nft, adj s23, ef s23
DVE consumes m=adj*ef (+r1 accum) in landing order (slots 2,3,0,1); ACT
does |m| (+r2 accum) for slots 2,3,0 and DVE computes the last slot's |m|
itself via max(m,-m) to skip one cross-engine hop.  The rowsum row (ones
column of the tT chain) is guarded into SBUF, PE-transposed to columns and
reciprocal'd as columns (128x4, ~60ns - a (1,512) row reciprocal costs
3.3us).  r1/r2 columns are normalized per slot, PE-transposed to rows, and
de-interleaved by one strided DVE copy.  The fused output matmul is split:
main part (W2.T@tT + W1.T@nft) runs early with its PSUM groups left open;
the rank-2 part (u0 (x) r1n + u1 (x) r2n) closes them on the critical tail.
leaky via ACT |.| + DVE combine, out DMA per half.
"""

import numpy as np

B, N, D, F = 8, 512, 64, 4
P = 128          # SBUF partitions
NT = N // P      # 4 interleave slots (row 4p+s <-> partition p, slot s)
HALF = N // 2
KMAIN = D + F    # 68: main fused matmul contraction (tT rows + nft rows)
NWARM = 8        # PE warm-up matmuls (HAM clock ramp) during DMA window
SLOPE = 0.01
C_A = (1.0 + SLOPE) / 2.0   # 0.505
C_B = (1.0 - SLOPE) / 2.0   # 0.495

# packed tensor column offsets
PK_PREV = 0                    # 4 slots x 65 cols of [prev | 1]
PK_W = PK_PREV + NT * (D + 1)  # 260: [W2.T ; W1.T] (68 rows used)
PK_U = PK_W + D                # 324: [u0 ; u1] (2 rows used)
PK_ID = PK_U + D               # 388: 128x128 identity
PK_COLS = PK_ID + P            # 516

# DVE/ACT consumption order: ACT-ring slots (2,3) land first, then SP's (0,1)
SLOT_ORDER = (2, 3, 0, 1)

_CACHE = {}


def _build_nc():
    import concourse.bacc as bacc
    import concourse.mybir as mybir

    FP32 = mybir.dt.float32
    BF16 = mybir.dt.bfloat16
    OP = mybir.AluOpType
    ACTF = mybir.ActivationFunctionType

    nc = bacc.Bacc("TRN2", target_bir_lowering=False)

    adj_d = nc.dram_tensor("adj", (N, N), FP32, kind="ExternalInput")
    adjt_d = nc.dram_tensor("adjt", (N, N), FP32, kind="ExternalInput")
    ef_d = nc.dram_tensor("ef", (N, N), FP32, kind="ExternalInput")
    packed_d = nc.dram_tensor("packed", (P, PK_COLS), FP32, kind="ExternalInput")
    nft_d = nc.dram_tensor("nft", (F, N), FP32, kind="ExternalInput")
    out_d = nc.dram_tensor("out", (D, N), FP32, kind="ExternalOutput")

    adj_sb = nc.alloc_sbuf_tensor("adj_sb", [P, NT, N], FP32)
    ef_sb = nc.alloc_sbuf_tensor("ef_sb", [P, NT, N], FP32)
    adjt_sb = nc.alloc_sbuf_tensor("adjt_sb", [P, NT, N], FP32)
    m_sb = nc.alloc_sbuf_tensor("m_sb", [P, NT, N], FP32)
    absm_sb = nc.alloc_sbuf_tensor("absm_sb", [P, NT, N], BF16)
    packed_sb = nc.alloc_sbuf_tensor("packed_sb", [P, PK_COLS], FP32)
    big_sb = nc.alloc_sbuf_tensor("big_sb", [KMAIN, N], FP32)
    rn_sb = nc.alloc_sbuf_tensor("rn_sb", [2, N], FP32)
    rn2_sb = nc.alloc_sbuf_tensor("rn2_sb", [P, NT, 2], FP32)
    rn2n_sb = nc.alloc_sbuf_tensor("rn2n_sb", [P, NT, 2], FP32)
    rg_sb = nc.alloc_sbuf_tensor("rg_sb", [D + 1, N], FP32)
    rc_sb = nc.alloc_sbuf_tensor("rc_sb", [P, NT], FP32)
    o1_sb = nc.alloc_sbuf_tensor("o1_sb", [D, N], FP32)
    outt_sb = nc.alloc_sbuf_tensor("outt_sb", [D, N], FP32)
    warm_sb = nc.alloc_sbuf_tensor("warm_sb", [P, P], FP32)

    warm_ps = nc.alloc_psum_tensor("warm_ps", [P, P], FP32)
    tTp = nc.alloc_psum_tensor("tTp", [D + 1, N], FP32)
    rsc_ps = nc.alloc_psum_tensor("rsc_ps", [P, NT], FP32)
    s8p = nc.alloc_psum_tensor("s8p", [2, N], FP32)
    xa0 = nc.alloc_psum_tensor("xa0", [D, HALF], FP32)
    xa1 = nc.alloc_psum_tensor("xa1", [D, HALF], FP32)

    # SBUF views into packed
    prev_v = packed_sb[:, PK_PREV:PK_W].rearrange("p (s e) -> p s e", s=NT)
    w2w1_v = packed_sb[0:KMAIN, PK_W:PK_W + D]
    uu_v = packed_sb[0:2, PK_U:PK_U + D]
    id_v = packed_sb[:, PK_ID:PK_ID + P]

    # guarded rowsum row on partition 64 (aligned with tTp's rowsum row);
    # stride-NT view: element (s, p) is rowsum[4p+s]
    rgrow_v = rg_sb[D:D + 1, :]
    rgrow_sub = rg_sb[D:D + 1, :].rearrange("o (p s) -> o s p", s=NT)

    s_adjt = [nc.alloc_semaphore(f"s_adjt{s}") for s in range(NT)]
    s_adj_a = nc.alloc_semaphore("s_adj_a")
    s_ef_a = nc.alloc_semaphore("s_ef_a")
    s_adj_b = nc.alloc_semaphore("s_adj_b")
    s_ef_b = nc.alloc_semaphore("s_ef_b")
    s_pk = nc.alloc_semaphore("s_pk")
    s_nft = nc.alloc_semaphore("s_nft")
    s_scr = nc.alloc_semaphore("s_scr")
    s_m = nc.alloc_semaphore("s_m")
    s_r2 = nc.alloc_semaphore("s_r2")
    s_tt = nc.alloc_semaphore("s_tt")
    s_rg = nc.alloc_semaphore("s_rg")
    s_rsc = nc.alloc_semaphore("s_rsc")
    s_rn = nc.alloc_semaphore("s_rn")
    s_s8 = nc.alloc_semaphore("s_s8")
    s_tcopy = nc.alloc_semaphore("s_tcopy")
    s_rncopy = nc.alloc_semaphore("s_rncopy")
    s_xa = nc.alloc_semaphore("s_xa")
    s_absx = nc.alloc_semaphore("s_absx")
    s_out = nc.alloc_semaphore("s_out")
    s_odma = nc.alloc_semaphore("s_odma")
    s_fin = nc.alloc_semaphore("s_fin")
    final_vals = [(s, 16) for s in s_adjt] + [
        (s_adj_a, 16), (s_ef_a, 16),
        (s_adj_b, 16), (s_ef_b, 16), (s_pk, 16), (s_nft, 16), (s_scr, 1),
        (s_m, 4), (s_r2, 3), (s_tt, 1), (s_rg, 1), (s_rsc, 1), (s_rn, 4),
        (s_s8, 1), (s_tcopy, 1), (s_rncopy, 2), (s_xa, 2), (s_absx, 2),
        (s_out, 2), (s_odma, 32), (s_fin, 5),
    ]

    # DRAM views: row 4p+s -> (p, slot-halves); a half = 4KB/partition
    adj_r = adj_d.rearrange("(p s) j -> p (s j)", p=P)
    adjt_r = adjt_d.rearrange("(s p) i -> s p i", s=NT)
    ef_r = ef_d.rearrange("(p s) j -> p (s j)", p=P)
    HJ = 2 * N  # one half = 2 slots

    def half(t, h):
        return t[:, 2 * h:2 * h + 2, :].rearrange("p s j -> p (s j)")

    with nc.Block(no_gpsimd_drain=True) as block:

        @block.sync
        def _(sync):
            # SP ring: adjt slabs first (block layout: row 128s+p, so the
            # fp32 tT chain contracts sequential k blocks - this matches the
            # CPU accumulation order closely and is what keeps the pointwise
            # rel-err comfortably under the gate), then adj/ef slots 0,1
            for s in range(NT):
                sync.dma_start(adjt_sb[:, s, :], adjt_r[s]).then_inc(s_adjt[s], 16)
            sync.dma_start(half(adj_sb, 0), adj_r[:, 0:HJ]).then_inc(s_adj_a, 16)
            sync.dma_start(half(ef_sb, 0), ef_r[:, 0:HJ]).then_inc(s_ef_a, 16)
            # output halves
            sync.wait_ge(s_out, 1)
            sync.dma_start(out_d[:, 0:HALF], outt_sb[:, 0:HALF]).then_inc(s_odma, 16)
            sync.wait_ge(s_out, 2)
            sync.dma_start(out_d[:, HALF:N], outt_sb[:, HALF:N]).then_inc(s_odma, 16)
            sync.wait_ge(s_odma, 32)
            sync.drain()
            sync.sem_inc(s_fin, 1)
            sync.wait_ge(s_fin, 5)
            for s, v in final_vals:
                sync.sem_clear(s)

        @block.gpsimd
        def _(gpsimd):
            gpsimd.memset(warm_sb[:], 0.0).then_inc(s_scr)
            gpsimd.sem_inc(s_fin, 1)
            gpsimd.wait_ge(s_fin, 5)

        @block.scalar
        def _(scalar):
            # ACT ring: small inputs, then the slot-23 halves of adj/ef
            scalar.dma_start(packed_sb[:], packed_d[:]).then_inc(s_pk, 16)
            scalar.dma_start(big_sb[D:KMAIN, :], nft_d[:]).then_inc(s_nft, 16)
            scalar.dma_start(half(adj_sb, 1), adj_r[:, HJ:2 * HJ]).then_inc(s_adj_b, 16)
            scalar.dma_start(half(ef_sb, 1), ef_r[:, HJ:2 * HJ]).then_inc(s_ef_b, 16)
            # off the critical path: copy (adj@prev).T from PSUM into big
            scalar.wait_ge(s_tt, 1)
            scalar.activation(big_sb[0:D, :], tTp[0:D, :],
                              ACTF.Copy).then_inc(s_tcopy)
            # r2 = rowsum(|m|) for slots 2,3,0 (fp32 accum; bf16 dst dummy)
            for i, s in enumerate(SLOT_ORDER[:3]):
                scalar.wait_ge(s_m, i + 1)
                scalar.activation(absm_sb[:, s, :], m_sb[:, s, :], ACTF.Abs,
                                  accum_out=rn2_sb[:, s, 1:2]).then_inc(s_r2)
            # 0.495*|x| halves of the final leaky
            scalar.wait_ge(s_xa, 1)
            scalar.activation(o1_sb[:, 0:HALF], xa0[:], ACTF.Abs,
                              scale=C_B).then_inc(s_absx)
            scalar.wait_ge(s_xa, 2)
            scalar.activation(o1_sb[:, HALF:N], xa1[:], ACTF.Abs,
                              scale=C_B).then_inc(s_absx)
            scalar.drain()
            scalar.sem_inc(s_fin, 1)
            scalar.wait_ge(s_fin, 5)

        @block.tensor
        def _(tensor):
            # HAM warm-up on zeroed scratch while input DMAs stream
            tensor.wait_ge(s_scr, 1)
            for w in range(NWARM):
                tensor.matmul(warm_ps[:], warm_sb[:], warm_sb[:],
                              start=True, stop=True)
            # tT = [(adj@prev).T ; rowsum(adj) row], accumulated per slot
            tensor.wait_ge(s_pk, 16)
            for s in range(NT):
                tensor.wait_ge(s_adjt[s], 16)
                mm = tensor.matmul(tTp[:], prev_v[:, s, :], adjt_sb[:, s, :],
                                   start=(s == 0), stop=(s == NT - 1))
            mm.then_inc(s_tt)
            # guarded rowsum row -> columns (1x1 id keeps base partition 64)
            tensor.wait_ge(s_rg, 1)
            for s in range(NT):
                mm = tensor.matmul(rsc_ps[:, s:s + 1], rgrow_sub[:, s, :],
                                   id_v[D:D + 1, D:D + 1], is_transpose=True,
                                   start=(s == 0), stop=(s == NT - 1))
            mm.then_inc(s_rsc)
            # main fused part (x1+x2).T: runs early, PSUM groups stay open
            tensor.wait_ge(s_tcopy, 1)
            tensor.wait_ge(s_nft, 16)
            tensor.matmul(xa0[:], w2w1_v[:], big_sb[:, 0:HALF],
                          start=True, stop=False)
            tensor.matmul(xa1[:], w2w1_v[:], big_sb[:, HALF:N],
                          start=True, stop=False)
            # hold the HAM clock while the r path finishes
            for gate in (1, 2, 3, 4):
                tensor.wait_ge(s_m, gate)
                tensor.matmul(warm_ps[:], warm_sb[:], warm_sb[:],
                              start=True, stop=True)
                tensor.matmul(warm_ps[:], warm_sb[:], warm_sb[:],
                              start=True, stop=True)
            # normalized [r1n|r2n] columns -> rows, slots in landing order
            for i, s in enumerate(SLOT_ORDER):
                tensor.wait_ge(s_rn, i + 1)
                mm = tensor.matmul(s8p[:, s * P:(s + 1) * P],
                                   rn2n_sb[:, s, :], id_v[:],
                                   is_transpose=True,
                                   start=(i == 0), stop=(i == NT - 1))
            mm.then_inc(s_s8)
            # rank-2 completion: += u0 (x) r1n + u1 (x) r2n, closes the group
            tensor.wait_ge(s_rncopy, 1)
            tensor.matmul(xa0[:], uu_v[:], rn_sb[:, 0:HALF],
                          start=False, stop=True).then_inc(s_xa)
            tensor.wait_ge(s_rncopy, 2)
            tensor.matmul(xa1[:], uu_v[:], rn_sb[:, HALF:N],
                          start=False, stop=True).then_inc(s_xa)
            tensor.sem_inc(s_fin, 1)
            tensor.wait_ge(s_fin, 5)

        @block.vector
        def _(vector):
            # guard rowsum into SBUF (copy+max in one (1,512) op)
            vector.wait_ge(s_tt, 1)
            vector.tensor_scalar_max(rgrow_v, tTp[D:D + 1, :], 1e-30)
            vector.drain()
            vector.sem_inc(s_rg, 1)
            # m = adj*ef with fused row-reduce r1, slots in landing order
            for i, s in enumerate(SLOT_ORDER):
                vector.wait_ge(s_adj_b if s >= 2 else s_adj_a, 16)
                vector.wait_ge(s_ef_b if s >= 2 else s_ef_a, 16)
                vector.scalar_tensor_tensor(
                    out=m_sb[:, s, :], in0=adj_sb[:, s, :], scalar=0.0,
                    in1=ef_sb[:, s, :], op0=OP.add, op1=OP.mult,
                    accum_out=rn2_sb[:, s, 0:1]).then_inc(s_m)
            # last slot's |m| on DVE (skips an ACT hop): |m| = max(-m, m)
            s_last = SLOT_ORDER[-1]
            vector.drain()
            vector.scalar_tensor_tensor(
                out=absm_sb[:, s_last, :], in0=m_sb[:, s_last, :], scalar=-1.0,
                in1=m_sb[:, s_last, :], op0=OP.mult, op1=OP.max,
                accum_out=rn2_sb[:, s_last, 1:2])
            # reciprocal on columns (128x4: ~60ns vs 3.3us for a 1x512 row)
            vector.wait_ge(s_rsc, 1)
            vector.reciprocal(rc_sb[:], rsc_ps[:])
            vector.drain()
            # normalize r1/r2 columns by rcp columns (per-partition scalars)
            for i, s in enumerate(SLOT_ORDER):
                if s != s_last:
                    vector.wait_ge(s_r2, i + 1)
                vector.tensor_scalar_mul(
                    rn2n_sb[:, s, :], rn2_sb[:, s, :],
                    rc_sb[:, s:s + 1]).then_inc(s_rn)
            # rn rows PSUM -> SBUF, de-interleaving (s p) -> (p s); column
            # halves so each rank-2 half can start as soon as its half lands
            vector.wait_ge(s_s8, 1)
            s8v = s8p.rearrange("c (s p) -> c s p", s=NT)
            vector.tensor_copy(
                rn_sb[:, 0:HALF].rearrange("c (p s) -> c s p", s=NT),
                s8v[:, :, 0:P // 2]).then_inc(s_rncopy)
            vector.tensor_copy(
                rn_sb[:, HALF:N].rearrange("c (p s) -> c s p", s=NT),
                s8v[:, :, P // 2:P]).then_inc(s_rncopy)
            # final leaky halves: out = 0.505*x + 0.495*|x|
            vector.wait_ge(s_absx, 1)
            vector.scalar_tensor_tensor(
                out=outt_sb[:, 0:HALF], in0=xa0[:], scalar=C_A,
                in1=o1_sb[:, 0:HALF], op0=OP.mult, op1=OP.add).then_inc(s_out)
            vector.wait_ge(s_absx, 2)
            vector.scalar_tensor_tensor(
                out=outt_sb[:, HALF:N], in0=xa1[:], scalar=C_A,
                in1=o1_sb[:, HALF:N], op0=OP.mult, op1=OP.add).then_inc(s_out)
            vector.sem_inc(s_fin, 1)
            vector.wait_ge(s_fin, 5)

    nc.compile()
    return nc


def get_nc():
    if "nc" not in _CACHE:
        _CACHE["nc"] = _build_nc()
    return _CACHE["nc"]


def make_in_maps(prev_embeddings, adj, node_features, edge_features,
                 W1, W2, W3, W4):
    f32 = np.float32
    w4 = np.asarray(W4, f32)[:, 0]
    W3 = np.asarray(W3, f32)
    w2w1 = np.zeros((KMAIN, D), f32)
    w2w1[0:D] = np.asarray(W2, f32).T
    w2w1[D:KMAIN] = np.asarray(W1, f32).T
    uu = np.stack([C_A * (W3 @ w4), C_B * (W3 @ np.abs(w4))]).astype(f32)
    prev_ext = np.ones((B, N, D + 1), f32)
    prev_ext[:, :, 0:D] = np.asarray(prev_embeddings, f32)
    in_maps = []
    for b in range(B):
        packed = np.zeros((P, PK_COLS), f32)
        # tT contraction row k = 128s+p lands at packed[p, s*65 : s*65+65]
        packed[:, PK_PREV:PK_W] = (
            prev_ext[b].reshape(NT, P, D + 1).transpose(1, 0, 2).reshape(P, -1))
        packed[0:KMAIN, PK_W:PK_W + D] = w2w1
        packed[0:2, PK_U:PK_U + D] = uu
        packed[:, PK_ID:PK_ID + P] = np.eye(P, dtype=f32)
        in_maps.append({
            "adj": np.ascontiguousarray(adj[b], f32),
            "adjt": np.ascontiguousarray(np.asarray(adj[b]).T),
            "ef": np.ascontiguousarray(edge_features[b], f32),
            "packed": packed,
            "nft": np.ascontiguousarray(np.asarray(node_features[b]).T),
        })
    return in_maps


def kernel(prev_embeddings, adj, node_features, edge_features,
           W1, W2, W3, W4, _trace=False, _trace_kwargs=None):
    from concourse.bass_utils import run_bass_kernel_spmd

    nc = get_nc()
    in_maps = make_in_maps(prev_embeddings, adj, node_features, edge_features,
                           W1, W2, W3, W4)
    res = run_bass_kernel_spmd(nc, in_maps, list(range(B)),
                               trace=_trace, **(_trace_kwargs or {}))
    _CACHE["last_result"] = res
    return np.stack([np.ascontiguousarray(res.results[b]["out"].T)
                     for b in range(B)])
